# revision 3
# baseline (speedup 1.0000x reference)
"""CoordinateDescentRouter kernel for 8 Trainium2 NeuronCores.

Problem: x[8, 8192, 2048] f32, routing_token[1, 2048] f32, num_tokens=1024.
  s = einsum("bnd,rd->bn", x, routing_token)
  scores = coor_descent(s, 50 iters, k=1152, eps=1.0)   # exp(s+a+b)
  top_k(scores, 1024) -> (selected_scores, selected_indices)

Sharding: data-parallel over batch B: one batch row per core, routing token
replicated.

Key structural facts used:
  * After the final iteration b = -relu(s+a), so scores = exp(s+a+b) are
    exactly 1.0 for every token with fl(s+a) >= 0 and exp(s+a) < 1 otherwise.
    With ~1150 selected tokens (> 1024 = k), top_k's tie-break (lower index
    first) makes the answer "the 1024 lowest token ids with fl(s+a) >= 0",
    and all selected scores (after the straight-through trick) exactly 1.0.
  * In the logsumexp, any shift M~ with sb <= M~ + eps works as well as the
    true max (it cancels mathematically); sb = s - relu(s+a_prev) <= ~(-a_prev)
    + 1ulp, so M~ = -a_prev avoids the cross-partition max entirely.
"""

import numpy as np

B, N, D = 8, 8192, 2048
P = 128
C = N // P  # 64 columns; token n <-> (p, c) with n = c*128 + p
NT = 1024
JF = 8  # NT = 128 * JF
N_ITERS = 50
FETCH_K_RATIO = 9.0 / 8.0
BIG = 10001.0

_EFF_K = np.float32(min(NT * FETCH_K_RATIO, N))
_LOGK = np.float32(np.log(_EFF_K))
_A1 = np.float32(_LOGK - np.float32(np.log(np.float32(N))))

_cache = {}


def _build():
    import concourse.bacc as bacc
    import concourse.tile as tile
    from concourse import mybir

    F32 = mybir.dt.float32
    I32 = mybir.dt.int32
    Alu = mybir.AluOpType
    Act = mybir.ActivationFunctionType

    nc = bacc.Bacc("TRN2", target_bir_lowering=False, debug=False, num_devices=8)

    x = nc.dram_tensor("x", [N, D], F32, kind="ExternalInput").ap()
    rbc = nc.dram_tensor("rbc", [P, D], F32, kind="ExternalInput").ap()
    tri = nc.dram_tensor("tri", [P, P], F32, kind="ExternalInput").ap()
    io128 = nc.dram_tensor("io128", [P, P], F32, kind="ExternalInput").ap()
    io8 = nc.dram_tensor("io8", [P, JF], F32, kind="ExternalInput").ap()
    val = nc.dram_tensor("val", [P, C], F32, kind="ExternalInput").ap()
    idx_out = nc.dram_tensor("idx", [P, JF], I32, kind="ExternalOutput").ap()
    ones_out = nc.dram_tensor("ones", [P, JF], F32, kind="ExternalOutput").ap()
    t_out = nc.dram_tensor("tout", [P, C], F32, kind="ExternalOutput").ap()

    with tile.TileContext(nc) as tc:
        with (
            tc.tile_pool(name="xin", bufs=3) as xpool,
            tc.tile_pool(name="prod", bufs=2) as prodpool,
            tc.tile_pool(name="small", bufs=1) as small,
            tc.tile_pool(name="psA", bufs=2, space="PSUM") as psA,
            tc.tile_pool(name="psB", bufs=1, space="PSUM") as psB,
        ):
            # ---- persistent small tiles ----
            s = small.tile([P, C], F32)
            sb = small.tile([P, C], F32)
            e = small.tile([P, C], F32)
            r = small.tile([P, C], F32)
            t = small.tile([P, C], F32)
            a_b = small.tile([P, 1], F32)
            lgT = small.tile([P, 1], F32)
            u = small.tile([P, 1], F32)
            S_p = small.tile([P, 1], F32)
            logk_t = small.tile([P, 1], F32)
            ones128 = small.tile([P, P], F32)
            rbc_t = small.tile([P, D], F32)
            tri_t = small.tile([P, P], F32)
            io128_t = small.tile([P, P], F32)
            io8_t = small.tile([P, JF], F32)
            val_t = small.tile([P, C], F32)

            nc.sync.dma_start(rbc_t[:], rbc[:])
            nc.sync.dma_start(tri_t[:], tri[:])
            nc.sync.dma_start(io128_t[:], io128[:])
            nc.sync.dma_start(io8_t[:], io8[:])
            nc.sync.dma_start(val_t[:], val[:])
            nc.vector.memset(ones128[:], 1.0)
            nc.vector.memset(a_b[:], float(_A1))
            nc.vector.memset(logk_t[:], float(_LOGK))

            # ---- phase A: matvec s[p, c] = sum_d x[c*128+p, d] * r[d] ----
            xr = x.rearrange("(t p) d -> t p d", p=P)
            for tt in range(C):
                xt = xpool.tile([P, D], F32)
                nc.sync.dma_start(xt[:], xr[tt])
                prod = prodpool.tile([P, D], F32)
                nc.vector.scalar_tensor_tensor(
                    out=prod[:],
                    in0=xt[:],
                    scalar=0.0,
                    in1=rbc_t[:],
                    op0=Alu.bypass,
                    op1=Alu.mult,
                    accum_out=s[:, tt : tt + 1],
                )

            # ---- phase B: 49 coordinate-descent iterations ----
            # state: a_b = a_1 (iteration 1 is closed-form), r = relu(s + a_1)
            nc.scalar.activation(r[:], s[:], Act.Relu, bias=a_b[:])
            for it in range(N_ITERS - 1):
                nc.vector.tensor_tensor(sb[:], s[:], r[:], op=Alu.subtract)
                # e = exp(sb + a_prev)  (shift M~ = -a_prev), S_p = row sums
                nc.scalar.activation(
                    e[:], sb[:], Act.Exp, bias=a_b[:], accum_out=S_p[:]
                )
                Sg = psA.tile([P, 1], F32, space="PSUM")
                nc.tensor.matmul(
                    Sg[:], lhsT=ones128[:], rhs=S_p[:], start=True, stop=True
                )
                nc.scalar.activation(lgT[:], Sg[:], Act.Ln)
                # lse = lgT + M~ = lgT - a_prev ; a_new = logk - lse
                nc.vector.tensor_scalar(u[:], lgT[:], a_b[:], None, op0=Alu.subtract)
                nc.vector.scalar_tensor_tensor(
                    out=a_b[:], in0=u[:], scalar=-1.0, in1=logk_t[:],
                    op0=Alu.mult, op1=Alu.add,
                )
                if it < N_ITERS - 2:
                    nc.scalar.activation(r[:], s[:], Act.Relu, bias=a_b[:])
            # t = fl(s + a_50): selection mask source
            nc.vector.tensor_scalar(t[:], s[:], a_b[:], None, op0=Alu.add)

            # ---- phase C: first-1024-selected extraction ----
            mask = small.tile([P, C], F32)
            nc.vector.tensor_scalar(mask[:], t[:], 0.0, None, op0=Alu.is_ge)
            incl = psB.tile([P, C], F32, space="PSUM")
            nc.tensor.matmul(incl[:], lhsT=tri_t[:], rhs=mask[:], start=True, stop=True)
            # column totals: ones-column matmul -> [1, C] at partition 0
            colsum_ps = psB.tile([1, C], F32, space="PSUM")
            nc.tensor.matmul(
                colsum_ps[:], lhsT=ones128[:, 0:1], rhs=mask[:], start=True, stop=True
            )
            colsum = small.tile([1, C], F32)
            nc.vector.tensor_copy(colsum[:], colsum_ps[:])
            cinc = small.tile([1, C], F32)
            nc.vector.tensor_tensor_scan(
                out=cinc[:], data0=colsum[:], data1=colsum[:], initial=0.0,
                op0=Alu.add, op1=Alu.bypass,
            )
            cexc = small.tile([1, C], F32)
            nc.vector.tensor_tensor(cexc[:], cinc[:], colsum[:], op=Alu.subtract)
            cpb = psB.tile([P, C], F32, space="PSUM")
            nc.tensor.matmul(
                cpb[:], lhsT=ones128[0:1, :], rhs=cexc[:], start=True, stop=True
            )
            # rank_eff = (incl - 1) + colprefix for selected, >= 10000 otherwise
            v = small.tile([P, C], F32)
            nc.vector.scalar_tensor_tensor(
                out=v[:], in0=mask[:], scalar=-BIG, in1=incl[:],
                op0=Alu.mult, op1=Alu.add,
            )
            rank = small.tile([P, C], F32)
            nc.vector.scalar_tensor_tensor(
                out=rank[:], in0=v[:], scalar=BIG - 1.0, in1=cpb[:],
                op0=Alu.add, op1=Alu.add,
            )
            ri = small.tile([P, C], I32)
            nc.vector.tensor_copy(ri[:], rank[:])
            rd8 = small.tile([P, C], I32)
            nc.vector.tensor_scalar(rd8[:], ri[:], 3, None, op0=Alu.logical_shift_right)
            rm8 = small.tile([P, C], I32)
            nc.vector.tensor_scalar(rm8[:], ri[:], 7, None, op0=Alu.bitwise_and)
            rd8f = small.tile([P, C], F32)
            nc.vector.tensor_copy(rd8f[:], rd8[:])
            rm8f = small.tile([P, C], F32)
            nc.vector.tensor_copy(rm8f[:], rm8[:])

            # A_all[p, (c, jp)] = [rd8f[p, c] == jp]
            A_all = small.tile([P, C * P], F32)
            nc.vector.tensor_tensor(
                A_all[:].rearrange("p (c j) -> p c j", j=P),
                rd8f[:, :, None].to_broadcast([P, C, P]),
                io128_t[:, None, :].to_broadcast([P, C, P]),
                op=Alu.is_equal,
            )
            # Bv[p, (c, jf)] = val[p, c] * [rm8f[p, c] == jf]
            B0 = small.tile([P, C * JF], F32)
            nc.vector.tensor_tensor(
                B0[:].rearrange("p (c j) -> p c j", j=JF),
                rm8f[:, :, None].to_broadcast([P, C, JF]),
                io8_t[:, None, :].to_broadcast([P, C, JF]),
                op=Alu.is_equal,
            )
            Bv = small.tile([P, C * JF], F32)
            nc.vector.tensor_tensor(
                Bv[:].rearrange("p (c j) -> p c j", j=JF),
                B0[:].rearrange("p (c j) -> p c j", j=JF),
                val_t[:, :, None].to_broadcast([P, C, JF]),
                op=Alu.mult,
            )
            pidx = psB.tile([P, JF], F32, space="PSUM")
            for tt in range(C):
                nc.tensor.matmul(
                    pidx[:],
                    lhsT=A_all[:, tt * P : (tt + 1) * P],
                    rhs=Bv[:, tt * JF : (tt + 1) * JF],
                    start=(tt == 0),
                    stop=(tt == C - 1),
                )
            idx_sb = small.tile([P, JF], I32)
            nc.vector.tensor_copy(idx_sb[:], pidx[:])
            ones_sb = small.tile([P, JF], F32)
            nc.vector.memset(ones_sb[:], 1.0)

            nc.sync.dma_start(idx_out[:], idx_sb[:])
            nc.sync.dma_start(ones_out[:], ones_sb[:])
            nc.sync.dma_start(t_out[:], t[:])

    nc.compile()
    return nc


def _get_nc():
    if "nc" not in _cache:
        _cache["nc"] = _build()
    return _cache["nc"]


def _consts():
    if "consts" in _cache:
        return _cache["consts"]
    tri = (np.arange(P)[:, None] <= np.arange(P)[None, :]).astype(np.float32)
    io128 = np.tile(np.arange(P, dtype=np.float32), (P, 1))
    io8 = np.ascontiguousarray(io128[:, :JF])
    val = (np.arange(C, dtype=np.float32)[None, :] * P
           + np.arange(P, dtype=np.float32)[:, None])
    _cache["consts"] = (tri, io128, io8, val.astype(np.float32))
    return _cache["consts"]


def _register_ntff_hook():
    """Best-effort registration of the axon NTFF profile hook (tracing only)."""
    try:
        import sys, types
        import antenv  # noqa: F401
        if "antenv.axon_hooks" not in sys.modules:
            mod = types.ModuleType("antenv.axon_hooks")
            mod._hook = None
            mod.set_axon_ntff_profile_hook = lambda h: setattr(mod, "_hook", h)
            mod.get_axon_ntff_profile_hook = lambda: mod._hook
            sys.modules["antenv.axon_hooks"] = mod
            antenv.axon_hooks = mod
        from antenv.axon_hooks import (
            get_axon_ntff_profile_hook,
            set_axon_ntff_profile_hook,
        )
        if get_axon_ntff_profile_hook() is None:
            from trn_agent_boot.trn_boot import _ntff_profile_via_ctypes
            set_axon_ntff_profile_hook(
                _ntff_profile_via_ctypes("/opt/axon/libaxon_pjrt.so")
            )
        return True
    except Exception as exc:  # pragma: no cover
        print(f"ntff hook registration failed: {exc}")
        return False


def _run(inputs, trace=False, tmpdir=None):
    from concourse import bass_utils

    x = np.ascontiguousarray(np.asarray(inputs["x"], dtype=np.float32))
    routing_token = np.asarray(inputs["routing_token"], dtype=np.float32)
    num_tokens = int(inputs["num_tokens"])
    assert x.shape == (B, N, D) and routing_token.shape == (1, D)
    assert num_tokens == NT, f"kernel hardcodes num_tokens={NT}, got {num_tokens}"

    nc = _get_nc()
    tri, io128, io8, val = _consts()
    rbc = np.ascontiguousarray(np.broadcast_to(routing_token, (P, D)))

    in_maps = [
        {"x": x[c], "rbc": rbc, "tri": tri, "io128": io128, "io8": io8, "val": val}
        for c in range(B)
    ]
    kw = {}
    if trace:
        _register_ntff_hook()
        kw = dict(trace=True, tmpdir=tmpdir)
    res = bass_utils.run_bass_kernel_spmd(nc, in_maps, core_ids=list(range(B)), **kw)

    scores = np.empty((B, NT), dtype=np.float32)
    indices = np.empty((B, NT), dtype=np.int32)
    for c in range(B):
        out = res.results[c]
        t = np.asarray(out["tout"], dtype=np.float32)  # [P, C]; token n=(c*128+p)
        t_flat = t.T.reshape(-1)
        count = int((t_flat >= 0.0).sum())
        if count >= NT:
            scores[c] = np.asarray(out["ones"]).reshape(-1)[:NT]
            indices[c] = np.asarray(out["idx"]).reshape(-1)[:NT]
        else:  # pragma: no cover - structural fallback, not expected to trigger
            sc = np.where(t_flat >= 0.0, np.float32(1.0),
                          np.exp(t_flat, dtype=np.float32))
            order = np.lexsort((np.arange(N), -sc))[:NT]
            ss = sc[order]
            scores[c] = ss + (np.float32(1.0) - ss)
            indices[c] = order.astype(np.int32)
    return (scores, indices), res


def kernel(**inputs):
    (scores, indices), _ = _run(inputs, trace=False)
    return scores, indices


# revision 6
# speedup vs baseline: 1.1191x; 1.1191x over previous
"""CoordinateDescentRouter kernel for 8 Trainium2 NeuronCores.

Problem: x[8, 8192, 2048] f32, routing_token[1, 2048] f32, num_tokens=1024.
  s = einsum("bnd,rd->bn", x, routing_token)
  scores = coor_descent(s, 50 iters, k=1152, eps=1.0)   # exp(s+a+b)
  top_k(scores, 1024) -> (selected_scores, selected_indices)

Sharding: data-parallel over batch B: one batch row per core, routing token
replicated.

Key structural facts used:
  * After the final iteration b = -relu(s+a), so scores = exp(s+a+b) are
    exactly 1.0 for every token with fl(s+a) >= 0 and exp(s+a) < 1 otherwise.
    With ~1150 selected tokens (> 1024 = k), top_k's tie-break (lower index
    first) makes the answer "the 1024 lowest token ids with fl(s+a) >= 0",
    and all selected scores (after the straight-through trick) exactly 1.0.
  * In the logsumexp, any shift M~ with sb <= M~ + eps works as well as the
    true max (it cancels mathematically); sb = s - relu(s+a_prev) <= ~(-a_prev)
    + 1ulp, so M~ = -a_prev avoids the cross-partition max entirely.
"""

import numpy as np

B, N, D = 8, 8192, 2048
P = 128
C = N // P  # 64 columns; token n <-> (p, c) with n = c*128 + p
NT = 1024
JF = 8  # NT = 128 * JF
N_ITERS = 50
FETCH_K_RATIO = 9.0 / 8.0
BIG = 10001.0

_EFF_K = np.float32(min(NT * FETCH_K_RATIO, N))
_LOGK = np.float32(np.log(_EFF_K))
_A1 = np.float32(_LOGK - np.float32(np.log(np.float32(N))))

_cache = {}


def _build():
    import concourse.bacc as bacc
    import concourse.tile as tile
    from concourse import mybir

    F32 = mybir.dt.float32
    I32 = mybir.dt.int32
    Alu = mybir.AluOpType
    Act = mybir.ActivationFunctionType

    nc = bacc.Bacc("TRN2", target_bir_lowering=False, debug=False, num_devices=8)

    x = nc.dram_tensor("x", [N, D], F32, kind="ExternalInput").ap()
    rbc = nc.dram_tensor("rbc", [P, D], F32, kind="ExternalInput").ap()
    tri = nc.dram_tensor("tri", [P, P], F32, kind="ExternalInput").ap()
    io128 = nc.dram_tensor("io128", [P, P], F32, kind="ExternalInput").ap()
    io8 = nc.dram_tensor("io8", [P, JF], F32, kind="ExternalInput").ap()
    val = nc.dram_tensor("val", [P, C], F32, kind="ExternalInput").ap()
    idx_out = nc.dram_tensor("idx", [P, JF], I32, kind="ExternalOutput").ap()
    ones_out = nc.dram_tensor("ones", [P, JF], F32, kind="ExternalOutput").ap()
    t_out = nc.dram_tensor("tout", [P, C], F32, kind="ExternalOutput").ap()

    with tile.TileContext(nc) as tc:
        with (
            tc.tile_pool(name="xin", bufs=3) as xpool,
            tc.tile_pool(name="prod", bufs=2) as prodpool,
            tc.tile_pool(name="small", bufs=1) as small,
            tc.tile_pool(name="psA", bufs=2, space="PSUM") as psA,
            tc.tile_pool(name="psB", bufs=1, space="PSUM") as psB,
        ):
            # ---- persistent small tiles ----
            s = small.tile([P, C], F32)
            sb = small.tile([P, C], F32)
            e = small.tile([P, C], F32)
            r = small.tile([P, C], F32)
            t = small.tile([P, C], F32)
            a_b = small.tile([P, 1], F32)
            lgT = small.tile([P, 1], F32)
            u = small.tile([P, 1], F32)
            S_p = small.tile([P, 1], F32)
            logk_t = small.tile([P, 1], F32)
            ones128 = small.tile([P, P], F32)
            rbc_t = small.tile([P, D], F32)
            tri_t = small.tile([P, P], F32)
            io128_t = small.tile([P, P], F32)
            io8_t = small.tile([P, JF], F32)
            val_t = small.tile([P, C], F32)

            # ---- phase A: matvec s[p, c] = sum_d x[c*128+p, d] * r[d] ----
            # rbc must be fetched before the x stream: the x-tile pool recycles
            # slots behind the stt consumers, which read rbc_t (FIFO DMA queue).
            nc.sync.dma_start(rbc_t[:], rbc[:])
            # 4 MiB DMAs: each brings FPD=4 n-tiles of [128, D].
            FPD = 4
            xr = x.rearrange("(t f p) d -> t p f d", p=P, f=FPD)
            xts = []
            for tt in range(C // FPD):
                xt = xpool.tile([P, FPD * D], F32)
                nc.sync.dma_start(
                    xt[:].rearrange("p (f d) -> p f d", f=FPD), xr[tt]
                )
                xts.append(xt)

            nc.sync.dma_start(tri_t[:], tri[:])
            nc.sync.dma_start(io128_t[:], io128[:])
            nc.sync.dma_start(io8_t[:], io8[:])
            nc.sync.dma_start(val_t[:], val[:])
            nc.vector.memset(ones128[:], 1.0)
            nc.vector.memset(a_b[:], float(_A1))
            nc.vector.memset(logk_t[:], float(_LOGK))

            for tt in range(C // FPD):
                xt = xts[tt]
                for f in range(FPD):
                    prod = prodpool.tile([P, D], F32)
                    nc.vector.scalar_tensor_tensor(
                        out=prod[:],
                        in0=xt[:, f * D : (f + 1) * D],
                        scalar=0.0,
                        in1=rbc_t[:],
                        op0=Alu.bypass,
                        op1=Alu.mult,
                        accum_out=s[:, tt * FPD + f : tt * FPD + f + 1],
                    )

            # ---- phase B: 49 coordinate-descent iterations ----
            # state: a_b = a_1 (iteration 1 is closed-form), r = relu(s + a_1).
            # relu on DVE: r = max(s + a, 0) — keeps ACT to {Exp, Ln} so both
            # table sets stay resident in the two HW table slots (no reloads).
            nc.vector.tensor_scalar(r[:], s[:], a_b[:], 0.0, op0=Alu.add, op1=Alu.max)
            for it in range(N_ITERS - 1):
                nc.vector.tensor_tensor(sb[:], s[:], r[:], op=Alu.subtract)
                # e = exp(sb + a_prev)  (shift M~ = -a_prev), S_p = row sums
                nc.scalar.activation(
                    e[:], sb[:], Act.Exp, bias=a_b[:], accum_out=S_p[:]
                )
                Sg = psA.tile([P, 1], F32, space="PSUM")
                nc.tensor.matmul(
                    Sg[:], lhsT=ones128[:], rhs=S_p[:], start=True, stop=True
                )
                nc.scalar.activation(lgT[:], Sg[:], Act.Ln)
                # lse = lgT + M~ = lgT - a_prev ; a_new = logk - lse
                nc.vector.tensor_scalar(u[:], lgT[:], a_b[:], None, op0=Alu.subtract)
                nc.vector.scalar_tensor_tensor(
                    out=a_b[:], in0=u[:], scalar=-1.0, in1=logk_t[:],
                    op0=Alu.mult, op1=Alu.add,
                )
                if it < N_ITERS - 2:
                    nc.vector.tensor_scalar(
                        r[:], s[:], a_b[:], 0.0, op0=Alu.add, op1=Alu.max
                    )
            # t = fl(s + a_50): selection mask source
            nc.vector.tensor_scalar(t[:], s[:], a_b[:], None, op0=Alu.add)

            # ---- phase C: first-1024-selected extraction ----
            mask = small.tile([P, C], F32)
            nc.vector.tensor_scalar(mask[:], t[:], 0.0, None, op0=Alu.is_ge)
            incl = psB.tile([P, C], F32, space="PSUM")
            nc.tensor.matmul(incl[:], lhsT=tri_t[:], rhs=mask[:], start=True, stop=True)
            # column totals: ones-column matmul -> [1, C] at partition 0
            colsum_ps = psB.tile([1, C], F32, space="PSUM")
            nc.tensor.matmul(
                colsum_ps[:], lhsT=ones128[:, 0:1], rhs=mask[:], start=True, stop=True
            )
            colsum = small.tile([1, C], F32)
            nc.vector.tensor_copy(colsum[:], colsum_ps[:])
            cinc = small.tile([1, C], F32)
            nc.vector.tensor_tensor_scan(
                out=cinc[:], data0=colsum[:], data1=colsum[:], initial=0.0,
                op0=Alu.add, op1=Alu.bypass,
            )
            cexc = small.tile([1, C], F32)
            nc.vector.tensor_tensor(cexc[:], cinc[:], colsum[:], op=Alu.subtract)
            cpb = psB.tile([P, C], F32, space="PSUM")
            nc.tensor.matmul(
                cpb[:], lhsT=ones128[0:1, :], rhs=cexc[:], start=True, stop=True
            )
            # rank_eff = (incl - 1) + colprefix for selected, >= 10000 otherwise
            v = small.tile([P, C], F32)
            nc.vector.scalar_tensor_tensor(
                out=v[:], in0=mask[:], scalar=-BIG, in1=incl[:],
                op0=Alu.mult, op1=Alu.add,
            )
            rank = small.tile([P, C], F32)
            nc.vector.scalar_tensor_tensor(
                out=rank[:], in0=v[:], scalar=BIG - 1.0, in1=cpb[:],
                op0=Alu.add, op1=Alu.add,
            )
            ri = small.tile([P, C], I32)
            nc.vector.tensor_copy(ri[:], rank[:])
            rd8 = small.tile([P, C], I32)
            nc.vector.tensor_scalar(rd8[:], ri[:], 3, None, op0=Alu.logical_shift_right)
            rm8 = small.tile([P, C], I32)
            nc.vector.tensor_scalar(rm8[:], ri[:], 7, None, op0=Alu.bitwise_and)
            rd8f = small.tile([P, C], F32)
            nc.vector.tensor_copy(rd8f[:], rd8[:])
            rm8f = small.tile([P, C], F32)
            nc.vector.tensor_copy(rm8f[:], rm8[:])

            # A_all[p, (c, jp)] = [rd8f[p, c] == jp]
            A_all = small.tile([P, C * P], F32)
            nc.vector.tensor_tensor(
                A_all[:].rearrange("p (c j) -> p c j", j=P),
                rd8f[:, :, None].to_broadcast([P, C, P]),
                io128_t[:, None, :].to_broadcast([P, C, P]),
                op=Alu.is_equal,
            )
            # Bv[p, (c, jf)] = val[p, c] * [rm8f[p, c] == jf]
            B0 = small.tile([P, C * JF], F32)
            nc.vector.tensor_tensor(
                B0[:].rearrange("p (c j) -> p c j", j=JF),
                rm8f[:, :, None].to_broadcast([P, C, JF]),
                io8_t[:, None, :].to_broadcast([P, C, JF]),
                op=Alu.is_equal,
            )
            Bv = small.tile([P, C * JF], F32)
            nc.vector.tensor_tensor(
                Bv[:].rearrange("p (c j) -> p c j", j=JF),
                B0[:].rearrange("p (c j) -> p c j", j=JF),
                val_t[:, :, None].to_broadcast([P, C, JF]),
                op=Alu.mult,
            )
            pidx = psB.tile([P, JF], F32, space="PSUM")
            for tt in range(C):
                nc.tensor.matmul(
                    pidx[:],
                    lhsT=A_all[:, tt * P : (tt + 1) * P],
                    rhs=Bv[:, tt * JF : (tt + 1) * JF],
                    start=(tt == 0),
                    stop=(tt == C - 1),
                )
            idx_sb = small.tile([P, JF], I32)
            nc.vector.tensor_copy(idx_sb[:], pidx[:])
            ones_sb = small.tile([P, JF], F32)
            nc.vector.memset(ones_sb[:], 1.0)

            nc.sync.dma_start(idx_out[:], idx_sb[:])
            nc.sync.dma_start(ones_out[:], ones_sb[:])
            nc.sync.dma_start(t_out[:], t[:])

    nc.compile()
    return nc


def _get_nc():
    if "nc" not in _cache:
        _cache["nc"] = _build()
    return _cache["nc"]


def _consts():
    if "consts" in _cache:
        return _cache["consts"]
    tri = (np.arange(P)[:, None] <= np.arange(P)[None, :]).astype(np.float32)
    io128 = np.tile(np.arange(P, dtype=np.float32), (P, 1))
    io8 = np.ascontiguousarray(io128[:, :JF])
    val = (np.arange(C, dtype=np.float32)[None, :] * P
           + np.arange(P, dtype=np.float32)[:, None])
    _cache["consts"] = (tri, io128, io8, val.astype(np.float32))
    return _cache["consts"]


def _register_ntff_hook():
    """Best-effort registration of the axon NTFF profile hook (tracing only)."""
    try:
        import sys, types
        import antenv  # noqa: F401
        if "antenv.axon_hooks" not in sys.modules:
            mod = types.ModuleType("antenv.axon_hooks")
            mod._hook = None
            mod.set_axon_ntff_profile_hook = lambda h: setattr(mod, "_hook", h)
            mod.get_axon_ntff_profile_hook = lambda: mod._hook
            sys.modules["antenv.axon_hooks"] = mod
            antenv.axon_hooks = mod
        from antenv.axon_hooks import (
            get_axon_ntff_profile_hook,
            set_axon_ntff_profile_hook,
        )
        if get_axon_ntff_profile_hook() is None:
            from trn_agent_boot.trn_boot import _ntff_profile_via_ctypes
            set_axon_ntff_profile_hook(
                _ntff_profile_via_ctypes("/opt/axon/libaxon_pjrt.so")
            )
        return True
    except Exception as exc:  # pragma: no cover
        print(f"ntff hook registration failed: {exc}")
        return False


def _run(inputs, trace=False, tmpdir=None):
    from concourse import bass_utils

    x = np.ascontiguousarray(np.asarray(inputs["x"], dtype=np.float32))
    routing_token = np.asarray(inputs["routing_token"], dtype=np.float32)
    num_tokens = int(inputs["num_tokens"])
    assert x.shape == (B, N, D) and routing_token.shape == (1, D)
    assert num_tokens == NT, f"kernel hardcodes num_tokens={NT}, got {num_tokens}"

    nc = _get_nc()
    tri, io128, io8, val = _consts()
    rbc = np.ascontiguousarray(np.broadcast_to(routing_token, (P, D)))

    in_maps = [
        {"x": x[c], "rbc": rbc, "tri": tri, "io128": io128, "io8": io8, "val": val}
        for c in range(B)
    ]
    kw = {}
    if trace:
        _register_ntff_hook()
        kw = dict(trace=True, tmpdir=tmpdir)
    res = bass_utils.run_bass_kernel_spmd(nc, in_maps, core_ids=list(range(B)), **kw)

    scores = np.empty((B, NT), dtype=np.float32)
    indices = np.empty((B, NT), dtype=np.int32)
    for c in range(B):
        out = res.results[c]
        t = np.asarray(out["tout"], dtype=np.float32)  # [P, C]; token n=(c*128+p)
        t_flat = t.T.reshape(-1)
        count = int((t_flat >= 0.0).sum())
        if count >= NT:
            scores[c] = np.asarray(out["ones"]).reshape(-1)[:NT]
            indices[c] = np.asarray(out["idx"]).reshape(-1)[:NT]
        else:  # pragma: no cover - structural fallback, not expected to trigger
            sc = np.where(t_flat >= 0.0, np.float32(1.0),
                          np.exp(t_flat, dtype=np.float32))
            order = np.lexsort((np.arange(N), -sc))[:NT]
            ss = sc[order]
            scores[c] = ss + (np.float32(1.0) - ss)
            indices[c] = order.astype(np.int32)
    return (scores, indices), res


def kernel(**inputs):
    (scores, indices), _ = _run(inputs, trace=False)
    return scores, indices
